# revision 9
# baseline (speedup 1.0000x reference)
"""Trainium2 Bass kernel for nn_AttentionLayer (B=64, L1=L2=512, H=A=1024).

Math (per batch b):
    P_lt = tanh(reps_lt[b] @ W) * diag_W        [L1, A]
    P_rt = tanh(reps_rt[b] @ W)                 [L2, A]
    S    = P_lt @ P_rt.T                        [L1, L2]
    out  = softmax(S, axis=-1)                  (masks are all-ones -> no-ops)

Distribution: data-parallel over batch across 8 NeuronCores (8 batches/core).

Layout strategy: the PE contracts over the partition dim, so both matmuls
want their operands in [contraction, free] layout.  We transpose
reps[b] -> [H, L] on the HOST while sharding, then:
    proj:   psum[a_chunk, l] += W[k_chunk, a_chunk].T @ XT[k_chunk, l]
            (lhsT = W natural layout, rhs = XT)  -> P.T in [A, L] layout
    scores: psum[l_chunk, r] += P_lt.T[a_chunk, l_chunk].T @ P_rt.T[a_chunk, r]
so no on-device transposes are needed anywhere.

Matmul operands are fed as float32r (fp32 bits, PE streams at 1 cycle/row for
free dim >= 256); tanh/exp/softmax run in fp32 on ACT/DVE.  Set USE_BF16=True
to switch the matmul datapath to bf16 (halves DMA + SBUF).
"""

from contextlib import ExitStack

import numpy as np

import concourse.bass as bass
import concourse.bacc as bacc
import concourse.mybir as mybir
import concourse.tile as tile
from concourse.bass_utils import run_bass_kernel_spmd

B, L, H, A = 64, 512, 1024, 1024
NCORES = 8
BPC = B // NCORES  # batches per core
PD = 128  # partition dim
KC = H // PD  # contraction chunks for proj
MC = A // PD  # att-dim chunks
LC = L // PD  # L1 chunks for scores

F32 = mybir.dt.float32
USE_BF16 = True

if USE_BF16:
    XDT = mybir.dt.bfloat16  # storage dtype of matmul operand tiles
    _X_NP_DT = "bfloat16"  # ml_dtypes name for host-side cast
else:
    # fp32 bits streamed as float32r: 1 cycle/row on the PE for N>=256.
    # walrus requires every on-chip producer feeding an fp32r matmul to
    # round its output to fp32r, so operand tiles carry the dtype.
    XDT = mybir.dt.float32r
    _X_NP_DT = "float32"


def _build_body(ctx: ExitStack, tc: "tile.TileContext", out, xt_lt, xt_rt, w, dw,
                repeat: int = 1):
    nc = tc.nc
    act = mybir.ActivationFunctionType

    wp = ctx.enter_context(tc.tile_pool(name="wpool", bufs=1))
    xp = ctx.enter_context(tc.tile_pool(name="xpool", bufs=2))
    pp = ctx.enter_context(tc.tile_pool(name="ppool", bufs=2))
    ep = ctx.enter_context(tc.tile_pool(name="epool", bufs=3))
    sp = ctx.enter_context(tc.tile_pool(name="spool", bufs=6))
    op = ctx.enter_context(tc.tile_pool(name="opool", bufs=2))
    ps_proj = ctx.enter_context(tc.tile_pool(name="psA", bufs=4, space="PSUM"))
    ps_scr = ctx.enter_context(tc.tile_pool(name="psB", bufs=3, space="PSUM"))

    # Resident weights: w_sb[p, k, a] = W[k*128 + p, a]
    w_sb = wp.tile([PD, KC, A], XDT)
    nc.sync.dma_start(out=w_sb, in_=w.rearrange("(k p) a -> p k a", p=PD))
    # Per-partition diagonal scale: dw_sb[p, m] = dw[m*128 + p]
    dw_sb = wp.tile([PD, MC], F32)
    nc.sync.dma_start(out=dw_sb, in_=dw.rearrange("(m p) -> p m", p=PD))

    for b in [bb for _ in range(repeat) for bb in range(BPC)]:
        # x tiles: [p, k, l] = XT[b, k*128+p, l]
        x_lt = xp.tile([PD, KC, L], XDT, tag="xlt")
        nc.sync.dma_start(out=x_lt, in_=xt_lt[b].rearrange("(k p) l -> p k l", p=PD))
        x_rt = xp.tile([PD, KC, L], XDT, tag="xrt")
        nc.sync.dma_start(out=x_rt, in_=xt_rt[b].rearrange("(k p) l -> p k l", p=PD))

        # Projections -> P.T tiles in [A, L] layout (m-chunk per tile).
        lt_p = []
        rt_p = []
        for m in range(MC):
            ps = ps_proj.tile([PD, L], F32, tag="psp")
            for k in range(KC):
                nc.tensor.matmul(
                    ps,
                    lhsT=w_sb[:, k, m * PD : (m + 1) * PD],
                    rhs=x_lt[:, k, :],
                    start=(k == 0),
                    stop=(k == KC - 1),
                )
            # tanh in-place on PSUM, then scale by diag_W into SBUF.
            nc.scalar.activation(ps, ps, act.Tanh)
            ltm = pp.tile([PD, L], XDT, tag=f"lt{m}")
            nc.vector.tensor_scalar_mul(ltm, ps, dw_sb[:, m : m + 1])
            lt_p.append(ltm)
        for m in range(MC):
            ps = ps_proj.tile([PD, L], F32, tag="psp")
            for k in range(KC):
                nc.tensor.matmul(
                    ps,
                    lhsT=w_sb[:, k, m * PD : (m + 1) * PD],
                    rhs=x_rt[:, k, :],
                    start=(k == 0),
                    stop=(k == KC - 1),
                )
            rtm = pp.tile([PD, L], XDT, tag=f"rt{m}")
            nc.scalar.activation(rtm, ps, act.Tanh)
            rt_p.append(rtm)

        # Scores + row softmax.
        o_sb = op.tile([PD, LC, L], F32, tag="o")
        for lm in range(LC):
            ps2 = ps_scr.tile([PD, L], F32, tag="pss")
            for m in range(MC):
                nc.tensor.matmul(
                    ps2,
                    lhsT=lt_p[m][:, lm * PD : (lm + 1) * PD],
                    rhs=rt_p[m],
                    start=(m == 0),
                    stop=(m == MC - 1),
                )
            nmx = sp.tile([PD, 1], F32, tag="nmx")
            nc.vector.reduce_max(nmx, ps2, axis=mybir.AxisListType.X, negate=True)
            ex = ep.tile([PD, L], F32, tag="ex")
            sm = sp.tile([PD, 1], F32, tag="sm")
            nc.scalar.activation(ex, ps2, act.Exp, bias=nmx, accum_out=sm)
            rc = sp.tile([PD, 1], F32, tag="rc")
            nc.vector.reciprocal(rc, sm)
            nc.vector.tensor_scalar_mul(o_sb[:, lm, :], ex, rc)
        nc.sync.dma_start(out=out[b].rearrange("(q p) r -> p q r", p=PD), in_=o_sb)


def build_nc(repeat: int = 1) -> "bacc.Bacc":
    nc = bacc.Bacc("TRN2", target_bir_lowering=False, debug=False, num_devices=NCORES)
    xt_lt = nc.dram_tensor("xt_lt", [BPC, H, L], XDT, kind="ExternalInput").ap()
    xt_rt = nc.dram_tensor("xt_rt", [BPC, H, L], XDT, kind="ExternalInput").ap()
    w = nc.dram_tensor("w", [H, A], XDT, kind="ExternalInput").ap()
    dw = nc.dram_tensor("dw", [A], F32, kind="ExternalInput").ap()
    out = nc.dram_tensor("out", [BPC, L, L], F32, kind="ExternalOutput").ap()
    with tile.TileContext(nc) as tc, ExitStack() as ctx:
        _build_body(ctx, tc, out, xt_lt, xt_rt, w, dw, repeat=repeat)
    nc.compile()
    return nc


_NC_CACHE = None


def _get_nc():
    global _NC_CACHE
    if _NC_CACHE is None:
        _NC_CACHE = build_nc()
    return _NC_CACHE


def _x_np(a):
    if _X_NP_DT == "float32":
        return np.ascontiguousarray(a, dtype=np.float32)
    import ml_dtypes

    return np.ascontiguousarray(a).astype(ml_dtypes.bfloat16)


def make_in_maps(reps_lt, reps_rt, attn_w1, diagonal_W):
    """Shard + lay out host-side: per-core [BPC, H, L] transposed inputs."""
    w = _x_np(np.asarray(attn_w1, dtype=np.float32))
    dw = np.ascontiguousarray(np.asarray(diagonal_W, dtype=np.float32).reshape(A))
    in_maps = []
    for c in range(NCORES):
        sl = slice(c * BPC, (c + 1) * BPC)
        xt_lt = _x_np(np.asarray(reps_lt[sl], dtype=np.float32).transpose(0, 2, 1))
        xt_rt = _x_np(np.asarray(reps_rt[sl], dtype=np.float32).transpose(0, 2, 1))
        in_maps.append({"xt_lt": xt_lt, "xt_rt": xt_rt, "w": w, "dw": dw})
    return in_maps


def kernel(reps_lt, reps_rt, mask_lt, mask_rt, attn_w1, diagonal_W):
    reps_lt = np.asarray(reps_lt, dtype=np.float32)
    reps_rt = np.asarray(reps_rt, dtype=np.float32)
    mask_lt = np.asarray(mask_lt, dtype=np.float32)
    mask_rt = np.asarray(mask_rt, dtype=np.float32)
    attn_w1 = np.asarray(attn_w1, dtype=np.float32)
    diagonal_W = np.asarray(diagonal_W, dtype=np.float32)

    if not (np.all(mask_lt == 1.0) and np.all(mask_rt == 1.0)):
        # General-mask fallback (never hit for this problem's all-ones masks):
        # multiplicative masking changes the softmax input, so compute on host.
        attn_lt = np.tanh(reps_lt @ attn_w1) * diagonal_W.reshape(1, 1, A)
        attn_rt = np.tanh(reps_rt @ attn_w1)
        s = np.einsum("bla,bra->blr", attn_lt, attn_rt)
        s = s * mask_lt[:, :, None] * mask_rt[:, None, :]
        e = np.exp(s - s.max(-1, keepdims=True))
        p = e / e.sum(-1, keepdims=True)
        return (p * mask_lt[:, :, None] * mask_rt[:, None, :]).astype(np.float32)

    nc = _get_nc()
    in_maps = make_in_maps(reps_lt, reps_rt, attn_w1, diagonal_W)
    res = run_bass_kernel_spmd(nc, in_maps, core_ids=list(range(NCORES)))
    out = np.concatenate([res.results[c]["out"] for c in range(NCORES)], axis=0)
    return np.ascontiguousarray(out.astype(np.float32))
